# revision 15
# baseline (speedup 1.0000x reference)
"""GCN 3-layer forward on 8 Trainium2 NeuronCores (Bass/Tile).

Self-contained: hardcodes the problem shapes from the spec.
kernel(**inputs) -> np.ndarray [50000, 128] float32.

Layout: feature-major ("transposed") on chip — features on partitions,
nodes along the free dim.  Nodes are degree-sorted and round-robin
assigned to cores; per-core slots sorted by A-half degree so the
segmented reduce is a short list of constant-degree runs shared by all
cores (histograms padded to a common shape).  Message gather uses
dma_gather(transpose=True) from f16 tables in DRAM; the int16 index
limit is handled by an A/B split of the table (cores 0-3 via view
[0,32768), cores 4-7 via [TR-32768,TR)).  The B-phase output is
permuted back to slot order with ap_gather over fp32-paired f16
columns.  Symmetric normalization is factorized: table rows pre-scaled
by dinv[src], aggregates post-scaled by dinv[dst].  Conv bias is
dropped (cancels exactly in training-mode BatchNorm).  BN+LeakyReLU is
one ScalarE activation; BN stats ride accum_out + a tiny AllReduce;
tables are shared with AllGather.
"""
import sys

sys.path.insert(0, "/opt/trn_rl_repo")

import numpy as np
import ml_dtypes

import concourse.bacc as bacc
import concourse.mybir as mybir
import concourse.tile as tile
from concourse.bass_utils import run_bass_kernel_spmd

F16 = np.float16

N, E, DIN, DH, DOUT = 50000, 800000, 128, 256, 128
EPS = 1e-5
SLOPE = 0.01
NCORES = 8
ACORES = 4          # cores 0..3 form the "A" half of the table
CH = 896            # max idxs per dma_gather call (HW SWDGE ring limit)
SUB = 4             # gather sub-calls per reduce chunk
CHB = CH * SUB      # edges per reduce chunk
IMAX = 32768        # int16 index window


# ----------------------------------------------------------------------------
# host-side schedule construction
# ----------------------------------------------------------------------------

def _wrap_idx(arr):
    """1-D int array -> [128, len/16] int16 wrapped+replicated layout."""
    L = len(arr)
    assert L % 16 == 0
    a = np.asarray(arr, np.int16).reshape(L // 16, 16).T  # [16, L/16]
    return np.tile(a, (8, 1)).copy()  # [128, L/16]


def _chunk_cols(gvec, cap):
    """Split columns into chunks of <= cap edges, breaking at even column
    indices only (keeps 4B alignment for the 2x DVE reduce mode)."""
    chunks = []
    lo = 0
    acc = 0
    j = 0
    n = len(gvec)
    while j < n:
        g2 = gvec[j] + (gvec[j + 1] if j + 1 < n else 0)
        if acc + g2 > cap and acc > 0:
            chunks.append((lo, j))
            lo = j
            acc = 0
        acc += g2
        j += 2
    chunks.append((lo, n))
    return chunks


def _runs_for_chunk(gvec, lo, hi):
    """Consecutive constant-degree runs (skipping g==0) within cols [lo,hi).
    Returns (in_off, out_col, n, g); in_off relative to chunk start."""
    runs = []
    off = 0
    j = lo
    while j < hi:
        g = gvec[j]
        k = j
        while k < hi and gvec[k] == g:
            k += 1
        if g > 0:
            runs.append((int(off), int(j), int(k - j), int(g)))
        off += g * (k - j)
        j = k
    return runs


def _phase_schedule(gvec, cap):
    chunks = _chunk_cols(gvec, cap)
    out = []
    for (lo, hi) in chunks:
        out.append({"cols": (lo, hi),
                    "nedges": int(np.sum(gvec[lo:hi])),
                    "runs": _runs_for_chunk(gvec, lo, hi)})
    return out


def _col_positions(sched, gvec, ch):
    pos = np.zeros(len(gvec), np.int64)
    for k, chk in enumerate(sched):
        lo, hi = chk["cols"]
        off = 0
        for j in range(lo, hi):
            pos[j] = k * ch + off
            off += gvec[j]
    return pos


def preprocess(edge_index, x):
    src0 = np.asarray(edge_index[0], np.int64)
    dst0 = np.asarray(edge_index[1], np.int64)
    loop = np.arange(N, dtype=np.int64)

    deg = np.bincount(np.concatenate([dst0, loop]), minlength=N)
    dinv = (1.0 / np.sqrt(deg.astype(np.float64))).astype(np.float32)

    order = np.argsort(deg, kind="stable")
    core_of = np.empty(N, np.int32)
    core_of[order] = np.arange(N) % NCORES

    # the appended self-loop is applied locally on-core; streams carry
    # only the original E edges (incl. accidental src==dst edges).
    src, dst = src0, dst0
    a_mask = core_of[src] < ACORES
    gA = np.bincount(dst[a_mask], minlength=N)
    gB = np.bincount(dst[~a_mask], minlength=N)
    gAp = gA + (gA & 1)   # even-padded phase degrees
    gBp = gB + (gB & 1)

    # per-core slot order: sorted by even A-degree
    gmaxA = int(gAp.max())
    counts_a = np.zeros((NCORES, gmaxA + 1), np.int64)
    percore_nodes = []
    for c in range(NCORES):
        nodes_c = np.flatnonzero(core_of == c)
        nodes_c = nodes_c[np.argsort(gAp[nodes_c], kind="stable")]
        percore_nodes.append(nodes_c)
        counts_a[c] = np.bincount(gAp[nodes_c], minlength=gmaxA + 1)

    mA = counts_a.max(axis=0)
    mA[0] += 2                        # guarantee pad slots (zero table rows)
    mA += mA & 1                      # even bucket sizes
    nc_raw = int(mA.sum())
    NC = -(-nc_raw // 512) * 512
    mA[0] += NC - nc_raw

    gvecA = np.repeat(np.arange(gmaxA + 1), mA)
    startA = np.concatenate([[0], np.cumsum(mA)])

    slot_node = []
    for c in range(NCORES):
        sn = np.full(NC, -1, np.int64)
        nodes_c = percore_nodes[c]
        gs = gAp[nodes_c]
        ranks = np.arange(len(nodes_c)) - np.searchsorted(gs, gs)
        sn[startA[gs] + ranks] = nodes_c
        slot_node.append(sn)

    slot_of = np.full(N, -1, np.int64)
    for c in range(NCORES):
        real = slot_node[c] >= 0
        slot_of[slot_node[c][real]] = np.flatnonzero(real)

    TR = 2 + NCORES * NC
    baseB = max(0, TR - IMAX)
    row_of = 1 + core_of.astype(np.int64) * NC + slot_of

    assert int(row_of[core_of < ACORES].max()) < IMAX
    assert int(row_of[core_of >= ACORES].min()) >= baseB

    # --- B phase: slots paired (2p, 2p+1); both columns padded to the
    # pair's max even B-degree so ap_gather can move fp32-paired columns.
    gB_slot = np.zeros((NCORES, NC), np.int64)
    for c in range(NCORES):
        real = slot_node[c] >= 0
        gB_slot[c][real] = gBp[slot_node[c][real]]
    pairdeg = gB_slot.reshape(NCORES, NC // 2, 2).max(axis=2)  # [cores, NP]
    NP = NC // 2
    gmaxB = int(pairdeg.max())
    counts_b = np.zeros((NCORES, gmaxB + 1), np.int64)
    for c in range(NCORES):
        counts_b[c] = np.bincount(pairdeg[c], minlength=gmaxB + 1)
    mB = counts_b.max(axis=0)
    npb_raw = int(mB.sum())
    NPB = -(-npb_raw // 16) * 16
    mB[0] += NPB - npb_raw
    NCB = 2 * NPB
    pairdegvec = np.repeat(np.arange(gmaxB + 1), mB)   # per B-pair degree
    gvecB = np.repeat(pairdegvec, 2)                   # per B-column degree
    startBp = np.concatenate([[0], np.cumsum(mB)])

    paircol = np.zeros((NCORES, NP), np.int64)  # slot-pair -> B-pair pos
    for c in range(NCORES):
        pd = pairdeg[c]
        o = np.argsort(pd, kind="stable")
        ranks = np.arange(NP) - np.searchsorted(pd[o], pd[o])
        pc = np.empty(NP, np.int64)
        pc[o] = startBp[pd[o]] + ranks
        paircol[c] = pc

    zA = int(startA[1])                   # row of (core 0, slot m0A-1)
    zB = int(7 * NC + startA[1])          # row of (core 7, slot m0A-1)
    schedA = _phase_schedule(gvecA, CHB)
    schedB = _phase_schedule(gvecB, CHB)
    schedA1 = _phase_schedule(gvecA, CH)   # layer-1: 896-edge chunks
    schedB1 = _phase_schedule(gvecB, CH)
    EA, EB = len(schedA) * CHB, len(schedB) * CHB
    EA1, EB1 = len(schedA1) * CH, len(schedB1) * CH

    colposA = _col_positions(schedA, gvecA, CHB)
    colposB = _col_positions(schedB, gvecB, CHB)
    colposA1 = _col_positions(schedA1, gvecA, CH)
    colposB1 = _col_positions(schedB1, gvecB, CH)

    def _streams(es, cols, default, base, specs):
        o = np.argsort(cols, kind="stable")
        es_, cols_ = es[o], cols[o]
        ranks = np.arange(len(cols_)) - np.searchsorted(cols_, cols_)
        outs = []
        for colpos, size in specs:
            s = np.full(size, default, np.int64)
            s[colpos[cols_] + ranks] = row_of[es_] - base
            assert s.min() >= 0 and s.max() < IMAX
            outs.append(_wrap_idx(s))
        return outs

    idxA, idxB, idxA1, idxB1 = [], [], [], []
    apgb_l, dinvb_l, xown_l = [], [], []
    xs = (np.asarray(x, np.float32) * dinv[:, None]).astype(F16)
    for c in range(NCORES):
        m = (core_of[dst] == c) & a_mask
        sA, sA1 = _streams(src[m], slot_of[dst[m]], zA, 0,
                           [(colposA, EA), (colposA1, EA1)])
        idxA.append(sA)
        idxA1.append(sA1)

        m = (core_of[dst] == c) & (~a_mask)
        sl = slot_of[dst[m]]
        cols = paircol[c][sl // 2] * 2 + (sl & 1)
        sB, sB1 = _streams(src[m], cols, zB - baseB, baseB,
                           [(colposB, EB), (colposB1, EB1)])
        idxB.append(sB)
        idxB1.append(sB1)

        apgb_l.append(_wrap_idx(paircol[c]))

        dv = np.zeros(NC, np.float32)
        real = slot_node[c] >= 0
        dv[real] = dinv[slot_node[c][real]]
        dinvb_l.append(np.tile(dv[None, :].astype(F16), (128, 1)))

        xo = np.zeros((128, NC), F16)
        xo[:, np.flatnonzero(real)] = xs[slot_node[c][real]].T
        xown_l.append(xo)

    xt = np.zeros((TR, DIN), F16)
    xt[row_of] = xs

    sched = {
        "NC": NC, "NCB": NCB, "NPB": NPB, "TR": TR, "baseB": baseB,
        "schedA": schedA, "schedB": schedB, "EA": EA, "EB": EB,
        "schedA1": schedA1, "schedB1": schedB1, "EA1": EA1, "EB1": EB1,
        "m0A": int(startA[1]), "m0B": int(2 * startBp[1]),
        "slot_node": slot_node,
    }
    data = {"xt": xt, "idxA": idxA, "idxB": idxB, "idxA1": idxA1,
            "idxB1": idxB1, "apgb": apgb_l, "dinvb": dinvb_l,
            "xown": xown_l}
    return sched, data


# ----------------------------------------------------------------------------
# device kernel builder
# ----------------------------------------------------------------------------

def build_nc(sched, debug=False):
    NC, NCB, TR = sched["NC"], sched["NCB"], sched["TR"]
    NPB = sched["NPB"]
    baseB = sched["baseB"]
    EA, EB = sched["EA"], sched["EB"]
    EA1, EB1 = sched["EA1"], sched["EB1"]
    fp32 = mybir.dt.float32
    f16 = mybir.dt.float16
    i16 = mybir.dt.int16
    AF = mybir.ActivationFunctionType
    OP = mybir.AluOpType
    NCH = NC // 512

    nc = bacc.Bacc("TRN2", target_bir_lowering=False, num_devices=NCORES,
                   num_swdge_queues=4)

    xt_d = nc.dram_tensor("xt", [TR, DIN], f16, kind="ExternalInput")
    idxa_d = nc.dram_tensor("idxa", [128, EA // 16], i16, kind="ExternalInput")
    idxb_d = nc.dram_tensor("idxb", [128, EB // 16], i16, kind="ExternalInput")
    idxa1_d = nc.dram_tensor("idxa1", [128, EA1 // 16], i16,
                             kind="ExternalInput")
    idxb1_d = nc.dram_tensor("idxb1", [128, EB1 // 16], i16,
                             kind="ExternalInput")
    xown_d = nc.dram_tensor("xown", [128, NC], f16, kind="ExternalInput")
    apgb_d = nc.dram_tensor("apgb", [128, NC // 32], i16, kind="ExternalInput")
    dinvb_d = nc.dram_tensor("dinvb", [128, NC], f16, kind="ExternalInput")
    w0_d = nc.dram_tensor("w0", [128, 256], f16, kind="ExternalInput")
    w1_d = nc.dram_tensor("w1", [128, 512], f16, kind="ExternalInput")
    w2_d = nc.dram_tensor("w2", [128, 256], f16, kind="ExternalInput")
    bnp_d = nc.dram_tensor("bnp", [128, 10], fp32, kind="ExternalInput")
    identb_d = nc.dram_tensor("identb", [128, 128], f16, kind="ExternalInput")
    identf_d = nc.dram_tensor("identf", [128, 128], fp32, kind="ExternalInput")
    out_d = nc.dram_tensor("out", [NC, DOUT], fp32, kind="ExternalOutput")
    dbg = {}
    if debug:
        for name, shape in [
            ("dbg_agg0", [128, NC]), ("dbg_cv0", [128, 2 * NC]),
            ("dbg_h1", [128, 2 * NC]), ("dbg_agg1", [128, 2 * NC]),
            ("dbg_st0", [128, 4]),
        ]:
            dbg[name] = nc.dram_tensor(name, shape, fp32,
                                       kind="ExternalOutput")

    with tile.TileContext(nc) as tc:
        with (
            tc.tile_pool(name="const", bufs=1) as constp,
            tc.tile_pool(name="gat", bufs=2) as gatp,
            tc.tile_pool(name="red", bufs=1) as redp,
            tc.tile_pool(name="agg", bufs=2) as aggp,
            tc.tile_pool(name="perm", bufs=1) as permp,
            tc.tile_pool(name="small", bufs=2) as smallp,
            tc.tile_pool(name="rowt", bufs=2) as rowp,
            tc.tile_pool(name="ps", bufs=3, space="PSUM") as psp,
            tc.tile_pool(name="pst", bufs=2, space="PSUM") as pstp,
            tc.tile_pool(name="pstf", bufs=2, space="PSUM") as pstfp,
            tc.tile_pool(name="psq", bufs=1, space="PSUM") as psqp,
            tc.tile_pool(name="dram", bufs=1, space="DRAM") as dramp,
        ):
            # ---- resident constants
            apgb = constp.tile([128, NC // 32], i16, tag="apgb")
            dinvb = constp.tile([128, NC], f16, tag="dinvb")
            w0 = constp.tile([128, 256], f16, tag="w0")
            w1 = constp.tile([128, 512], f16, tag="w1")
            w2 = constp.tile([128, 256], f16, tag="w2")
            bnp = constp.tile([128, 10], fp32, tag="bnp")
            identb = constp.tile([128, 128], f16, tag="identb")
            identf = constp.tile([128, 128], fp32, tag="identf")
            xown = constp.tile([128, NC], f16, tag="xown")
            idxa_t = constp.tile([128, EA // 16], i16, tag="idxa")
            idxb_t = constp.tile([128, EB // 16], i16, tag="idxb")
            idxa1_t = constp.tile([128, EA1 // 16], i16, tag="idxa1")
            idxb1_t = constp.tile([128, EB1 // 16], i16, tag="idxb1")
            for t, d in [(apgb, apgb_d),
                         (dinvb, dinvb_d), (w0, w0_d), (w1, w1_d),
                         (w2, w2_d), (bnp, bnp_d), (identb, identb_d),
                         (identf, identf_d), (xown, xown_d),
                         (idxa_t, idxa_d), (idxb_t, idxb_d),
                         (idxa1_t, idxa1_d), (idxb1_t, idxb1_d)]:
                nc.sync.dma_start(t[:], d[:])


            tbl = dramp.tile([TR, DH], f16, tag="tbl", addr_space="Shared")
            tbl2 = dramp.tile([TR, DOUT], f16, tag="tbl2",
                              addr_space="Shared")
            agsrc = dramp.tile([NC, DH], f16, tag="agsrc")
            agsrc2 = dramp.tile([NC, DOUT], f16, tag="agsrc2")

            def gather_reduce(table, elem, blocks):
                """A+B gather phases -> (outA f16, outB f16) [128,2,*].

                blocks==1: elem-128 gathers, 3584-edge chunks x 4 subcalls.
                blocks==2: single 512B gathers, 896-edge chunks; one reduce
                covers both feature blocks via a 4D AP."""
                outA = redp.tile([128, 2, NC], f16, tag="outA")
                outB = redp.tile([128, 2, NCB], f16, tag="outB")
                qn = [0]
                with nc.allow_low_precision(reason="DVE accumulates fp32"):
                    for phase in ("A", "B"):
                        if phase == "A":
                            outX, m0 = outA, sched["m0A"]
                            view = table[0:min(TR, IMAX), :]
                            idxt = idxa1_t if blocks == 2 else idxa_t
                            sch = (sched["schedA1"] if blocks == 2
                                   else sched["schedA"])
                        else:
                            outX, m0 = outB, sched["m0B"]
                            view = table[baseB:TR, :]
                            idxt = idxb1_t if blocks == 2 else idxb_t
                            sch = (sched["schedB1"] if blocks == 2
                                   else sched["schedB"])
                        for j in range(blocks):
                            if m0 > 0:
                                nc.vector.memset(outX[:, j, :m0], 0)
                        for k, chk in enumerate(sch):
                            if blocks == 2:
                                gb = gatp.tile([128, 2, CH], f16, tag="gb2")
                                nc.gpsimd.dma_gather(
                                    out_ap=gb[:], in_ap=view,
                                    idxs_ap=idxt[:, k * (CH // 16):
                                                 (k + 1) * (CH // 16)],
                                    num_idxs=CH, num_idxs_reg=CH,
                                    elem_size=256, transpose=True,
                                    queue_num=qn[0] % 4,
                                )
                                qn[0] += 1
                                for (ioff, ocol, n, g) in chk["runs"]:
                                    nc.vector.tensor_reduce(
                                        outX[:, :, ocol:ocol + n],
                                        gb[:, :, ioff:ioff + n * g]
                                        .rearrange("p b (n g) -> p b n g",
                                                   g=g),
                                        axis=mybir.AxisListType.X,
                                        op=OP.add,
                                    )
                            else:
                                gb = gatp.tile([128, 1, CHB], f16, tag="gb0")
                                for sx in range(SUB):
                                    nc.gpsimd.dma_gather(
                                        out_ap=gb[:, :,
                                                  sx * CH:(sx + 1) * CH],
                                        in_ap=view,
                                        idxs_ap=idxt[:, k * (CHB // 16)
                                                     + sx * (CH // 16):
                                                     k * (CHB // 16)
                                                     + (sx + 1) * (CH // 16)],
                                        num_idxs=CH,
                                        num_idxs_reg=CH,
                                        elem_size=128,
                                        transpose=True,
                                        queue_num=(qn[0] + sx) % 4,
                                    )
                                qn[0] += SUB
                                for (ioff, ocol, n, g) in chk["runs"]:
                                    nc.vector.tensor_reduce(
                                        outX[:, 0, ocol:ocol + n],
                                        gb[:, 0, ioff:ioff + n * g]
                                        .rearrange("p (n g) -> p n g", g=g),
                                        axis=mybir.AxisListType.X,
                                        op=OP.add,
                                    )
                return outA, outB

            def merge(outA, outB, blocks, own):
                """B-perm + add + local self-loop + dinv[dst] scale.

                own: list of per-block [128, NC] f16 APs of this core's
                dinv-prescaled rows (self-loop contribution = dinv*own)."""
                aggT = aggp.tile([128, 2, NC], f16, tag="aggbuf")
                for j in range(blocks):
                    tmp = permp.tile([128, NC // 2], fp32, tag="ptmp")
                    nc.gpsimd.ap_gather(
                        out_ap=tmp[:],
                        in_ap=outB[:, j, :].bitcast(fp32),
                        idxs_ap=apgb[:],
                        channels=128,
                        num_elems=NPB,
                        d=1,
                        num_idxs=NC // 2,
                    )
                    tb = tmp[:].bitcast(f16)
                    nc.vector.tensor_tensor(aggT[:, j, :], tb,
                                            outA[:, j, :], OP.add)
                    nc.vector.tensor_tensor(aggT[:, j, :], aggT[:, j, :],
                                            own[j], OP.add)
                    nc.vector.tensor_tensor(aggT[:, j, :], aggT[:, j, :],
                                            dinvb[:], OP.mult)
                return aggT

            def bn_consts(st, blocks_out, bn_off, layer):
                """AllReduce stats -> per-feature scale A / bias B tiles."""
                stin = dramp.tile([128, 4], fp32, tag=f"stin{layer}")
                stout = dramp.tile([128, 4], fp32, tag=f"stout{layer}",
                                   addr_space="Shared")
                nc.gpsimd.dma_start(stin[:], st[:])
                nc.gpsimd.collective_compute(
                    "AllReduce", OP.add,
                    replica_groups=[list(range(NCORES))],
                    ins=[stin.opt()], outs=[stout.opt()],
                )
                stg = smallp.tile([128, 4], fp32, tag="stg")
                nc.sync.dma_start(stg[:], stout[:])
                b = blocks_out
                mu = smallp.tile([128, 2], fp32, tag="mu")
                va = smallp.tile([128, 2], fp32, tag="va")
                Ab = smallp.tile([128, 2], fp32, tag="Ab")
                Bb = smallp.tile([128, 2], fp32, tag="Bb")
                musq = smallp.tile([128, 2], fp32, tag="musq")
                rstd = smallp.tile([128, 2], fp32, tag="rstd")
                nc.vector.tensor_scalar(mu[:, :b], stg[:, 0:b], 1.0 / N, None,
                                        op0=OP.mult)
                nc.vector.tensor_scalar(va[:, :b], stg[:, 2:2 + b], 1.0 / N,
                                        None, op0=OP.mult)
                nc.vector.tensor_tensor(musq[:, :b], mu[:, :b], mu[:, :b],
                                        OP.mult)
                nc.vector.tensor_tensor(va[:, :b], va[:, :b], musq[:, :b],
                                        OP.subtract)
                sqv = smallp.tile([128, 2], fp32, tag="sqv")
                nc.vector.tensor_scalar(sqv[:, :b], va[:, :b], EPS, None,
                                        op0=OP.add)
                nc.scalar.activation(sqv[:, :b], sqv[:, :b], AF.Sqrt)
                nc.vector.reciprocal(rstd[:, :b], sqv[:, :b])
                gsl = bnp[:, bn_off:bn_off + b]
                bsl = bnp[:, bn_off + b:bn_off + 2 * b]
                nc.vector.tensor_tensor(Ab[:, :b], rstd[:, :b], gsl, OP.mult)
                nc.vector.tensor_tensor(Bb[:, :b], mu[:, :b], Ab[:, :b],
                                        OP.mult)
                nc.vector.tensor_tensor(Bb[:, :b], bsl, Bb[:, :b],
                                        OP.subtract)
                return Ab, Bb

            def conv_bn(aggT, wt, KT, bn_off, lrelu, layer):
                """matmul (out 2 blocks of 128) + BN(+lrelu) -> h f16."""
                cv = aggp.tile([128, 2, NC], f16, tag="aggbuf")
                ssum = smallp.tile([128, 2, NCH], fp32, tag="ssum")
                sqsum = smallp.tile([128, 2, NCH], fp32, tag="sqsum")
                for j in range(2):
                    for t in range(NCH):
                        ps = psp.tile([128, 512], fp32, tag="cps")
                        sl = slice(t * 512, (t + 1) * 512)
                        for kt in range(KT):
                            lhsT = wt[:, kt * 256 + j * 128:
                                      kt * 256 + (j + 1) * 128]
                            nc.tensor.matmul(ps[:], lhsT, aggT[:, kt, sl],
                                             start=(kt == 0),
                                             stop=(kt == KT - 1))
                        sq = smallp.tile([128, 512], f16, tag="sqd")
                        nc.scalar.activation(cv[:, j, sl], ps[:], AF.Copy,
                                             accum_out=ssum[:, j, t:t + 1])
                        nc.scalar.activation(sq[:], ps[:], AF.Square,
                                             accum_out=sqsum[:, j, t:t + 1])
                st = smallp.tile([128, 4], fp32, tag="stl")
                for j in range(2):
                    nc.vector.tensor_reduce(st[:, j:j + 1], ssum[:, j, :],
                                            axis=mybir.AxisListType.X,
                                            op=OP.add)
                    nc.vector.tensor_reduce(st[:, 2 + j:3 + j], sqsum[:, j, :],
                                            axis=mybir.AxisListType.X,
                                            op=OP.add)
                Ab, Bb = bn_consts(st, 2, bn_off, layer)
                h = aggp.tile([128, 2, NC], f16, tag="aggbuf")
                fn = AF.Lrelu if lrelu else AF.Identity
                for j in range(2):
                    nc.scalar.activation(h[:, j, :], cv[:, j, :], fn,
                                         bias=Bb[:, j:j + 1],
                                         scale=Ab[:, j:j + 1], alpha=SLOPE)
                return h, cv, st

            def write_rows(srcT, blocks, dst_dram, width, prescale):
                """(optional dinv[src] prescale) + transpose + DMA rows."""
                if prescale:
                    hs = aggp.tile([128, 2, NC], f16, tag="aggbuf")
                    for j in range(blocks):
                        nc.vector.tensor_tensor(hs[:, j, :], srcT[:, j, :],
                                                dinvb[:], OP.mult)
                    srcT = hs
                for t in range(NC // 128):
                    row = rowp.tile([128, width], f16, tag="rowt")
                    for j in range(blocks):
                        pt = pstp.tile([128, 128], f16, tag="tps")
                        nc.tensor.transpose(
                            pt[:], srcT[:, j, t * 128:(t + 1) * 128],
                            identb[:])
                        nc.scalar.activation(row[:, j * 128:(j + 1) * 128],
                                             pt[:], AF.Copy)
                    nc.sync.dma_start(dst_dram[t * 128:(t + 1) * 128, :],
                                      row[:])
                return srcT

            # ================= layer 0 =================
            outA, outB = gather_reduce(xt_d, DIN, 1)
            aggT = merge(outA, outB, 1, [xown[:]])
            if debug:
                nc.gpsimd.dma_start(dbg["dbg_agg0"][:], aggT[:, 0, :])
            h1, cv0, st0 = conv_bn(aggT, w0, 1, 0, True, 0)
            if debug:
                nc.sync.dma_start(dbg["dbg_st0"][:], st0[:])
                nc.gpsimd.dma_start(
                    dbg["dbg_cv0"][:].rearrange("p (a b) -> p a b", a=2),
                    cv0[:])
                nc.gpsimd.dma_start(
                    dbg["dbg_h1"][:].rearrange("p (a b) -> p a b", a=2),
                    h1[:])
            hs1 = write_rows(h1, 2, agsrc, DH, prescale=True)
            nc.gpsimd.collective_compute(
                "AllGather", OP.bypass,
                replica_groups=[list(range(NCORES))],
                ins=[agsrc.opt()], outs=[tbl[1:1 + NCORES * NC, :]],
            )

            # ================= layer 1 =================
            outA, outB = gather_reduce(tbl, DH, 2)
            aggT = merge(outA, outB, 2, [hs1[:, 0, :], hs1[:, 1, :]])
            if debug:
                nc.gpsimd.dma_start(
                    dbg["dbg_agg1"][:].rearrange("p (a b) -> p a b", a=2),
                    aggT[:])
            h2, _, _ = conv_bn(aggT, w1, 2, 4, True, 1)
            # transform-first for layer 2: T2 = W2 @ (dinv * h2)
            hs2 = aggp.tile([128, 2, NC], f16, tag="aggbuf")
            for j in range(2):
                nc.vector.tensor_tensor(hs2[:, j, :], h2[:, j, :], dinvb[:],
                                        OP.mult)
            t2 = aggp.tile([128, 2, NC], f16, tag="aggbuf")
            for t in range(NCH):
                ps = psp.tile([128, 512], fp32, tag="cps")
                sl = slice(t * 512, (t + 1) * 512)
                for kt in range(2):
                    nc.tensor.matmul(ps[:], w2[:, kt * 128:(kt + 1) * 128],
                                     hs2[:, kt, sl],
                                     start=(kt == 0), stop=(kt == 1))
                nc.scalar.activation(t2[:, 0, sl], ps[:], AF.Copy)
            write_rows(t2, 1, agsrc2, DOUT, prescale=False)
            nc.gpsimd.collective_compute(
                "AllGather", OP.bypass,
                replica_groups=[list(range(NCORES))],
                ins=[agsrc2.opt()], outs=[tbl2[1:1 + NCORES * NC, :]],
            )

            # ================= layer 2 =================
            outA, outB = gather_reduce(tbl2, DOUT, 1)
            aggT = merge(outA, outB, 1, [t2[:, 0, :]])
            # aggT IS the conv output (transform-first); BN only, no lrelu.
            ssum = smallp.tile([128, 2, NCH], fp32, tag="ssum")
            sqsum = smallp.tile([128, 2, NCH], fp32, tag="sqsum")
            for t in range(NCH):
                sl = slice(t * 512, (t + 1) * 512)
                sq = smallp.tile([128, 512], f16, tag="sqd")
                nc.scalar.activation(sq[:], aggT[:, 0, sl], AF.Square,
                                     accum_out=sqsum[:, 0, t:t + 1])
                nc.vector.tensor_reduce(ssum[:, 0, t:t + 1], aggT[:, 0, sl],
                                        axis=mybir.AxisListType.X, op=OP.add)
            st = smallp.tile([128, 4], fp32, tag="stl")
            nc.vector.tensor_reduce(st[:, 0:1], ssum[:, 0, :],
                                    axis=mybir.AxisListType.X, op=OP.add)
            nc.vector.tensor_reduce(st[:, 2:3], sqsum[:, 0, :],
                                    axis=mybir.AxisListType.X, op=OP.add)
            nc.vector.memset(st[:, 1:2], 0)
            nc.vector.memset(st[:, 3:4], 0)
            Ab, Bb = bn_consts(st, 1, 8, 2)
            for t in range(NC // 128):
                hf = smallp.tile([128, 128], fp32, tag="hfin")
                nc.scalar.activation(hf[:], aggT[:, 0, t * 128:(t + 1) * 128],
                                     AF.Identity,
                                     bias=Bb[:, 0:1], scale=Ab[:, 0:1])
                row = rowp.tile([128, DOUT], fp32, tag="rowtf")
                pt = pstfp.tile([128, 128], fp32, tag="tpsf")
                nc.tensor.transpose(pt[:], hf[:], identf[:])
                nc.vector.tensor_copy(row[:], pt[:])
                nc.sync.dma_start(out_d[t * 128:(t + 1) * 128, :], row[:])

    nc.compile()
    return nc


# ----------------------------------------------------------------------------
# entry point
# ----------------------------------------------------------------------------

def _make_inmaps(sched, data, W0, W1, W2, g0, be0, g1, be1, g2, be2):
    w0 = np.ascontiguousarray(W0.T.astype(F16))
    w1 = np.ascontiguousarray(
        W1.T.reshape(2, 128, 256).transpose(1, 0, 2).reshape(128, 512)
        .astype(F16))
    w2 = np.ascontiguousarray(
        W2.T.reshape(2, 128, 128).transpose(1, 0, 2).reshape(128, 256)
        .astype(F16))
    bnp = np.zeros((128, 10), np.float32)
    bnp[:, 0:2] = g0.reshape(2, 128).T
    bnp[:, 2:4] = be0.reshape(2, 128).T
    bnp[:, 4:6] = g1.reshape(2, 128).T
    bnp[:, 6:8] = be1.reshape(2, 128).T
    bnp[:, 8] = g2
    bnp[:, 9] = be2
    identb = np.eye(128, dtype=F16)
    identf = np.eye(128, dtype=np.float32)
    maps = []
    for c in range(NCORES):
        maps.append({
            "xt": data["xt"], "idxa": data["idxA"][c],
            "idxb": data["idxB"][c], "idxa1": data["idxA1"][c],
            "idxb1": data["idxB1"][c], "apgb": data["apgb"][c],
            "dinvb": data["dinvb"][c], "xown": data["xown"][c],
            "w0": w0, "w1": w1, "w2": w2, "bnp": bnp,
            "identb": identb, "identf": identf,
        })
    return maps


_CACHE = {}


def kernel(x, edge_index, W0, b0, g0, be0, W1, b1, g1, be1, W2, b2, g2, be2,
           _trace=False, _tmpdir=None, _debug=False):
    x = np.asarray(x, np.float32)
    edge_index = np.asarray(edge_index, np.int32)
    args = [np.asarray(a, np.float32)
            for a in (W0, b0, g0, be0, W1, b1, g1, be1, W2, b2, g2, be2)]
    (W0, b0, g0, be0, W1, b1, g1, be1, W2, b2, g2, be2) = args
    # conv bias cancels exactly in training-mode BatchNorm -> ignored.

    key = (edge_index.tobytes()[:256], int(edge_index.sum()), bool(_debug))
    if key not in _CACHE:
        sched, data = preprocess(edge_index, x)
        nc_obj = build_nc(sched, debug=_debug)
        _CACHE[key] = (sched, nc_obj)
    else:
        sched, nc_obj = _CACHE[key]
        _, data = preprocess(edge_index, x)

    in_maps = _make_inmaps(sched, data, W0, W1, W2, g0, be0, g1, be1, g2, be2)
    res = run_bass_kernel_spmd(nc_obj, in_maps, core_ids=list(range(NCORES)),
                               trace=_trace, tmpdir=_tmpdir)

    out = np.zeros((N, DOUT), np.float32)
    for c in range(NCORES):
        o = np.asarray(res.results[c]["out"])
        sn = sched["slot_node"][c]
        real = sn >= 0
        out[sn[real]] = o[real]
    kernel._last_result = res
    kernel._last_sched = sched
    return out

